# revision 34
# baseline (speedup 1.0000x reference)
"""Trainium2 Bass kernel for a debiased GRU cell.

Computation (per batch row):
    r   = sigmoid(W_r @ [x; h] + b_r)
    u   = sigmoid(W_u @ [x; h] + b_u)
    hh  = tanh(W_h @ [x_int; r*h] + b_h)
    s   = score * u
    out = (1 - s) * hh + s * h

Strategy: data-parallel over 8 cores (8192 rows each). On-chip layout is
feature-major ([H, batch]) so that
  - activations never need an on-chip transpose (host supplies x.T / h.T),
  - gate biases fuse into the ACT engine's per-partition bias operand,
  - matmuls run with full K=128 / M=128 / N=512 tiles (PE at peak rate).
The only broadcast needed (attention score along partitions) runs on the
otherwise-idle GPSIMD engine. Output is produced as out.T and un-transposed
on the host.
"""

import numpy as np

import concourse.bacc as bacc
import concourse.bass as bass
import concourse.mybir as mybir
import concourse.tile as tile
from concourse.bass_utils import run_bass_kernel_spmd

B = 65536
I = 256
H = 256
NCORES = 8
BC = B // NCORES  # rows per core
NB = 512          # batch columns per block (max fp32 matmul free dim)
NBLK = BC // NB   # 16
FP32 = mybir.dt.float32
AF = mybir.ActivationFunctionType

_NC_CACHE = {}


def _build_nc(reps=1, loop=None, mm_dtype="fp32r",
              pg_bufs=6, ph_bufs=2, in_bufs=4, work_bufs=3, psum_fine=True,
              out_queue="scalar", split_loads=False):
    nc = bacc.Bacc(
        "TRN2",
        target_bir_lowering=False,
        debug=False,
        enable_asserts=False,
    )

    # Matmul-operand dtype. float32r streams fp32 bits through the PE at
    # full rate (1 cycle/row vs 4 for plain fp32); walrus requires every
    # producer of an fp32r-matmul operand to declare an fp32r output, so
    # the whole feeding path (DRAM tensor -> DMA -> SBUF tile -> matmul)
    # is declared float32r. Bit layout is identical to fp32.
    MDT = {"fp32": mybir.dt.float32, "fp32r": mybir.dt.float32r}[mm_dtype]

    xT = nc.dram_tensor("xT", [2 * I, BC], MDT, kind="ExternalInput")
    hT = nc.dram_tensor("hT", [H, BC], MDT, kind="ExternalInput")
    sc = nc.dram_tensor("sc", [NBLK, 1, NB], FP32, kind="ExternalInput")
    wg = nc.dram_tensor("wg", [128, 24 * 128], MDT, kind="ExternalInput")
    wh = nc.dram_tensor("wh", [128, 8 * 128], MDT, kind="ExternalInput")
    bg = nc.dram_tensor("bg", [128, 4], FP32, kind="ExternalInput")
    bh = nc.dram_tensor("bh", [128, 2], FP32, kind="ExternalInput")
    outT = nc.dram_tensor("outT", [H, BC], FP32, kind="ExternalOutput")

    # [blk, partition, k-chunk, col]
    xTr = xT.rearrange("(k p) (b n) -> b p k n", p=128, n=NB)
    hTr = hT.rearrange("(k p) (b n) -> b p k n", p=128, n=NB)
    outTr = outT.rearrange("(m p) (b n) -> b p m n", p=128, n=NB)

    with tile.TileContext(nc) as tc:
        with (
            tc.tile_pool(name="const", bufs=1) as cpool,
            tc.tile_pool(name="xin", bufs=in_bufs) as xpool,
            tc.tile_pool(name="hin", bufs=in_bufs) as hpool,
            tc.tile_pool(name="sin", bufs=in_bufs) as spool,
            tc.tile_pool(name="gates", bufs=work_bufs) as gpool,
            tc.tile_pool(name="work", bufs=work_bufs) as wpool,
            tc.tile_pool(name="outp", bufs=work_bufs) as opool,
            tc.tile_pool(name="psg", bufs=pg_bufs, space=bass.MemorySpace.PSUM) as pgpool,
            tc.tile_pool(name="psh", bufs=ph_bufs, space=bass.MemorySpace.PSUM) as phpool,
        ):
            # Gate weights split per gate-half so the first gate chain only
            # waits on its own 300 KB slice, not the full 2.1 MB weight load.
            wg_sb = cpool.tile([128, 24 * 128], MDT)
            for gi in range(4):
                nc.sync.dma_start(wg_sb[:, gi * 768:(gi + 1) * 768],
                                  wg[:, gi * 768:(gi + 1) * 768])
            bg_sb = cpool.tile([128, 4], FP32)
            nc.sync.dma_start(bg_sb[:], bg[:])
            wh_sb = cpool.tile([128, 8 * 128], MDT)
            nc.sync.dma_start(wh_sb[:], wh[:])
            bh_sb = cpool.tile([128, 2], FP32)
            nc.sync.dma_start(bh_sb[:], bh[:])

            def emit_gates(b):
                """Load block b, run gate matmuls + sigmoids + r*h."""
                xt = xpool.tile([128, 4, NB], MDT, tag="xt")
                if split_loads:
                    # two half-loads: the gate chains only wait on the half
                    # they read next, hiding more DMA latency inside a block
                    nc.sync.dma_start(xt[:, 0:2, :], xTr[b][:, 0:2, :])
                    nc.sync.dma_start(xt[:, 2:4, :], xTr[b][:, 2:4, :])
                else:
                    nc.sync.dma_start(xt[:], xTr[b])
                ht = hpool.tile([128, 2, NB], MDT, tag="ht")
                nc.sync.dma_start(ht[:], hTr[b])
                srow = spool.tile([1, NB], FP32, tag="srow")
                nc.sync.dma_start(srow[:], sc[b])
                sbc = spool.tile([128, 2, NB], FP32, tag="sbc")
                nc.gpsimd.partition_broadcast(sbc[:, 0, :], srow[:])
                nc.gpsimd.partition_broadcast(sbc[:, 1, :], srow[:])

                if psum_fine:
                    pgs = [pgpool.tile([128, NB], FP32, tag="pg", name=f"pg{b}_{i}") for i in range(4)]
                else:
                    pg_r = pgpool.tile([128, 2, NB], FP32, tag="pg")
                    pg_u = pgpool.tile([128, 2, NB], FP32, tag="pg")
                    pgs = [pg_r[:, 0, :], pg_r[:, 1, :], pg_u[:, 0, :], pg_u[:, 1, :]]
                for gi in range(4):  # r0, r1, u0, u1
                    dst = pgs[gi][:] if psum_fine else pgs[gi]
                    for k in range(6):
                        act = xt[:, k, :] if k < 4 else ht[:, k - 4, :]
                        c = gi * 6 + k
                        nc.tensor.matmul(
                            dst,
                            wg_sb[:, c * 128:(c + 1) * 128],
                            act,
                            start=(k == 0),
                            stop=(k == 5),
                        )
                r = gpool.tile([128, 2, NB], FP32, tag="r")
                u = gpool.tile([128, 2, NB], FP32, tag="u")
                for m in range(2):
                    nc.scalar.activation(
                        r[:, m, :], pgs[m][:] if psum_fine else pgs[m],
                        AF.Sigmoid, bias=bg_sb[:, m:m + 1]
                    )
                    nc.scalar.activation(
                        u[:, m, :], pgs[2 + m][:] if psum_fine else pgs[2 + m],
                        AF.Sigmoid, bias=bg_sb[:, 2 + m:3 + m]
                    )
                rh = wpool.tile([128, 2, NB], MDT, tag="rh")
                nc.vector.tensor_mul(rh[:], r[:], ht[:])
                # e2 = score*u and A = h*e2 only depend on the gate phase, so
                # they run here, off the post-tanh critical tail.
                e2 = wpool.tile([128, 2, NB], FP32, tag="e2")
                nc.vector.tensor_mul(e2[:], u[:], sbc[:])
                A = wpool.tile([128, 2, NB], FP32, tag="A")
                nc.vector.tensor_mul(A[:], ht[:], e2[:])
                return dict(b=b, xt=xt, rh=rh, e2=e2, A=A)

            def emit_h(st):
                b = st["b"]
                """h_hat matmul + tanh + final combine + store for block b."""
                if psum_fine:
                    phs = [phpool.tile([128, NB], FP32, tag="ph", name=f"ph{b}_{i}") for i in range(2)]
                else:
                    ph = phpool.tile([128, 2, NB], FP32, tag="ph")
                    phs = [ph[:, 0, :], ph[:, 1, :]]
                for m in range(2):
                    for k in range(4):
                        act = st["xt"][:, k, :] if k < 2 else st["rh"][:, k - 2, :]
                        c = m * 4 + k
                        nc.tensor.matmul(
                            phs[m][:] if psum_fine else phs[m],
                            wh_sb[:, c * 128:(c + 1) * 128],
                            act,
                            start=(k == 0),
                            stop=(k == 3),
                        )
                hhat = wpool.tile([128, 2, NB], FP32, tag="hhat")
                for m in range(2):
                    nc.scalar.activation(
                        hhat[:, m, :], phs[m][:] if psum_fine else phs[m],
                        AF.Tanh, bias=bh_sb[:, m:m + 1]
                    )
                # out = A - (e2-1)*hh  ==  hh + e2*(h - hh), with A = h*e2
                C = wpool.tile([128, 2, NB], FP32, tag="C")
                nc.vector.scalar_tensor_tensor(
                    C[:], st["e2"][:], 1.0, hhat[:],
                    op0=mybir.AluOpType.subtract, op1=mybir.AluOpType.mult,
                )
                o = opool.tile([128, 2, NB], FP32, tag="o")
                nc.vector.tensor_sub(o[:], st["A"][:], C[:])
                # store on the ACT HWDGE ring so it doesn't queue behind the
                # input loads on the SP ring
                out_eng = nc.scalar if out_queue == "scalar" else nc.sync
                out_eng.dma_start(outTr[b], o[:])

            # Software-pipelined emission: block b's h-chain is emitted after
            # block b+1's gate matmuls so the PE never waits on the r*h
            # elementwise product. reps>1 repeats the whole pass (same
            # output) — used only for slope-based timing in bench.py.
            def emit_pass():
                prev = None
                for _rep in range(reps):
                    for b in range(NBLK):
                        st = emit_gates(b)
                        if prev is not None:
                            emit_h(prev)
                        prev = st
                emit_h(prev)

            if loop is None:
                emit_pass()
            else:
                # bench-only: repeat the whole pass `loop` times inside one
                # NEFF execution for slope-based timing.
                with tc.For_i(0, loop, 1):
                    emit_pass()

    nc.compile()
    return nc


def _get_nc():
    if "nc" not in _NC_CACHE:
        _NC_CACHE["nc"] = _build_nc()
    return _NC_CACHE["nc"]


def _pack_weights(W_r, W_u, W_h, b_r, b_u, b_h):
    wg = np.empty((128, 24 * 128), np.float32)
    for gi in range(4):
        W = W_r if gi < 2 else W_u
        m = gi % 2
        for k in range(6):
            c = gi * 6 + k
            wg[:, c * 128:(c + 1) * 128] = W[m * 128:(m + 1) * 128,
                                             k * 128:(k + 1) * 128].T
    wh = np.empty((128, 8 * 128), np.float32)
    for m in range(2):
        for k in range(4):
            c = m * 4 + k
            wh[:, c * 128:(c + 1) * 128] = W_h[m * 128:(m + 1) * 128,
                                               k * 128:(k + 1) * 128].T
    bg = np.stack([b_r[:128], b_r[128:], b_u[:128], b_u[128:]], axis=1)
    bh = np.stack([b_h[:128], b_h[128:]], axis=1)
    return (np.ascontiguousarray(wg), np.ascontiguousarray(wh),
            np.ascontiguousarray(bg), np.ascontiguousarray(bh))


def _make_in_maps(inputs, h_prev, attention_score, W_r, b_r, W_u, b_u, W_h, b_h):
    inputs = np.asarray(inputs, np.float32)
    h_prev = np.asarray(h_prev, np.float32)
    attention_score = np.asarray(attention_score, np.float32)
    wg, wh, bg, bh = _pack_weights(
        np.asarray(W_r, np.float32), np.asarray(W_u, np.float32),
        np.asarray(W_h, np.float32), np.asarray(b_r, np.float32),
        np.asarray(b_u, np.float32), np.asarray(b_h, np.float32),
    )
    in_maps = []
    for c in range(NCORES):
        sl = slice(c * BC, (c + 1) * BC)
        in_maps.append({
            "xT": np.ascontiguousarray(inputs[sl].T),
            "hT": np.ascontiguousarray(h_prev[sl].T),
            "sc": np.ascontiguousarray(attention_score[sl].reshape(NBLK, 1, NB)),
            "wg": wg, "wh": wh, "bg": bg, "bh": bh,
        })
    return in_maps


def _run(in_maps, trace=False, **kwargs):
    return run_bass_kernel_spmd(
        _get_nc(), in_maps, core_ids=list(range(NCORES)), trace=trace, **kwargs
    )


def _gather(results):
    out = np.empty((B, H), np.float32)
    for c in range(NCORES):
        out[c * BC:(c + 1) * BC] = results[c]["outT"].T
    return out


def kernel(**inputs):
    res = _run(_make_in_maps(**inputs), trace=False)
    return _gather(res.results)


# revision 43
# speedup vs baseline: 1.0221x; 1.0221x over previous
"""Trainium2 Bass kernel for a debiased GRU cell.

Computation (per batch row):
    r   = sigmoid(W_r @ [x; h] + b_r)
    u   = sigmoid(W_u @ [x; h] + b_u)
    hh  = tanh(W_h @ [x_int; r*h] + b_h)
    s   = score * u
    out = (1 - s) * hh + s * h

Strategy: data-parallel over 8 cores (8192 rows each). On-chip layout is
feature-major ([H, batch]) so that
  - activations never need an on-chip transpose (host supplies x.T / h.T),
  - gate biases fuse into the ACT engine's per-partition bias operand,
  - matmuls run with full K=128 / M=128 / N=512 tiles (PE at peak rate).
The only broadcast needed (attention score along partitions) runs on the
otherwise-idle GPSIMD engine. Output is produced as out.T and un-transposed
on the host.
"""

import numpy as np

import concourse.bacc as bacc
import concourse.bass as bass
import concourse.mybir as mybir
import concourse.tile as tile
from concourse.bass_utils import run_bass_kernel_spmd

B = 65536
I = 256
H = 256
NCORES = 8
BC = B // NCORES  # rows per core
NB = 512          # batch columns per block (max fp32 matmul free dim)
NBLK = BC // NB   # 16
FP32 = mybir.dt.float32
AF = mybir.ActivationFunctionType

_NC_CACHE = {}


def _build_nc(reps=1, loop=None, mm_dtype="fp32r",
              pg_bufs=6, ph_bufs=2, in_bufs=4, work_bufs=3, psum_fine=True,
              out_queue="scalar", split_loads=True, group=1):
    nc = bacc.Bacc(
        "TRN2",
        target_bir_lowering=False,
        debug=False,
        enable_asserts=False,
    )

    # Matmul-operand dtype. float32r streams fp32 bits through the PE at
    # full rate (1 cycle/row vs 4 for plain fp32); walrus requires every
    # producer of an fp32r-matmul operand to declare an fp32r output, so
    # the whole feeding path (DRAM tensor -> DMA -> SBUF tile -> matmul)
    # is declared float32r. Bit layout is identical to fp32.
    MDT = {"fp32": mybir.dt.float32, "fp32r": mybir.dt.float32r}[mm_dtype]

    xT = nc.dram_tensor("xT", [2 * I, BC], MDT, kind="ExternalInput")
    hT = nc.dram_tensor("hT", [H, BC], MDT, kind="ExternalInput")
    sc = nc.dram_tensor("sc", [NBLK, 1, NB], FP32, kind="ExternalInput")
    wg = nc.dram_tensor("wg", [128, 24 * 128], MDT, kind="ExternalInput")
    wh = nc.dram_tensor("wh", [128, 8 * 128], MDT, kind="ExternalInput")
    bg = nc.dram_tensor("bg", [128, 4], FP32, kind="ExternalInput")
    bh = nc.dram_tensor("bh", [128, 2], FP32, kind="ExternalInput")
    outT = nc.dram_tensor("outT", [H, BC], FP32, kind="ExternalOutput")

    # [blk, partition, k-chunk, col] — DMA at `group`-block granularity
    GNB = group * NB
    xTr = xT.rearrange("(k p) (b n) -> b p k n", p=128, n=GNB)
    hTr = hT.rearrange("(k p) (b n) -> b p k n", p=128, n=GNB)
    scr = sc.rearrange("b o n -> b o n") if group == 1 else \
        sc.rearrange("(g j) o n -> g o (j n)", j=group)
    outTr = outT.rearrange("(m p) (b n) -> b p m n", p=128, n=GNB)

    with tile.TileContext(nc) as tc:
        with (
            tc.tile_pool(name="const", bufs=1) as cpool,
            tc.tile_pool(name="xin", bufs=in_bufs) as xpool,
            tc.tile_pool(name="hin", bufs=in_bufs) as hpool,
            tc.tile_pool(name="sin", bufs=in_bufs) as spool,
            tc.tile_pool(name="gates", bufs=work_bufs) as gpool,
            tc.tile_pool(name="work", bufs=work_bufs) as wpool,
            tc.tile_pool(name="outp", bufs=work_bufs) as opool,
            tc.tile_pool(name="psg", bufs=pg_bufs, space=bass.MemorySpace.PSUM) as pgpool,
            tc.tile_pool(name="psh", bufs=ph_bufs, space=bass.MemorySpace.PSUM) as phpool,
        ):
            # Gate weights split per gate-half so the first gate chain only
            # waits on its own 300 KB slice, not the full 2.1 MB weight load.
            wg_sb = cpool.tile([128, 24 * 128], MDT)
            for gi in range(4):
                nc.sync.dma_start(wg_sb[:, gi * 768:(gi + 1) * 768],
                                  wg[:, gi * 768:(gi + 1) * 768])
            bg_sb = cpool.tile([128, 4], FP32)
            nc.sync.dma_start(bg_sb[:], bg[:])
            wh_sb = cpool.tile([128, 8 * 128], MDT)
            nc.sync.dma_start(wh_sb[:], wh[:])
            bh_sb = cpool.tile([128, 2], FP32)
            nc.sync.dma_start(bh_sb[:], bh[:])

            def load_group(g):
                """DMA the inputs for blocks [g*group, (g+1)*group) in one
                burst each, plus the group-wide output staging tile."""
                xt = xpool.tile([128, 4, GNB], MDT, tag="xt")
                if split_loads:
                    # two half-loads: the gate chains only wait on the half
                    # they read next, hiding more DMA latency inside a block
                    nc.sync.dma_start(xt[:, 0:2, :], xTr[g][:, 0:2, :])
                    nc.sync.dma_start(xt[:, 2:4, :], xTr[g][:, 2:4, :])
                else:
                    nc.sync.dma_start(xt[:], xTr[g])
                ht = hpool.tile([128, 2, GNB], MDT, tag="ht")
                nc.sync.dma_start(ht[:], hTr[g])
                srow = spool.tile([1, GNB], FP32, tag="srow")
                nc.sync.dma_start(srow[:], scr[g])
                sbc = spool.tile([128, 2, GNB], FP32, tag="sbc")
                nc.gpsimd.partition_broadcast(sbc[:, 0, :], srow[:])
                nc.gpsimd.partition_broadcast(sbc[:, 1, :], srow[:])
                og = opool.tile([128, 2, GNB], FP32, tag="o")
                return dict(g=g, xt=xt, ht=ht, sbc=sbc, og=og)

            def emit_gates(grp, j):
                """Gate matmuls + sigmoids + r*h for sub-block j of a group."""
                b = grp["g"] * group + j
                js = slice(j * NB, (j + 1) * NB)
                xt = grp["xt"][:, :, js]
                ht = grp["ht"][:, :, js]

                if psum_fine:
                    pgs = [pgpool.tile([128, NB], FP32, tag="pg", name=f"pg{b}_{i}") for i in range(4)]
                else:
                    pg_r = pgpool.tile([128, 2, NB], FP32, tag="pg")
                    pg_u = pgpool.tile([128, 2, NB], FP32, tag="pg")
                    pgs = [pg_r[:, 0, :], pg_r[:, 1, :], pg_u[:, 0, :], pg_u[:, 1, :]]
                for gi in range(4):  # r0, r1, u0, u1
                    dst = pgs[gi][:] if psum_fine else pgs[gi]
                    for k in range(6):
                        act = xt[:, k, :] if k < 4 else ht[:, k - 4, :]
                        c = gi * 6 + k
                        nc.tensor.matmul(
                            dst,
                            wg_sb[:, c * 128:(c + 1) * 128],
                            act,
                            start=(k == 0),
                            stop=(k == 5),
                        )
                r = gpool.tile([128, 2, NB], FP32, tag="r")
                u = gpool.tile([128, 2, NB], FP32, tag="u")
                for m in range(2):
                    nc.scalar.activation(
                        r[:, m, :], pgs[m][:] if psum_fine else pgs[m],
                        AF.Sigmoid, bias=bg_sb[:, m:m + 1]
                    )
                    nc.scalar.activation(
                        u[:, m, :], pgs[2 + m][:] if psum_fine else pgs[2 + m],
                        AF.Sigmoid, bias=bg_sb[:, 2 + m:3 + m]
                    )
                rh = wpool.tile([128, 2, NB], MDT, tag="rh")
                nc.vector.tensor_mul(rh[:], r[:], ht)
                # e2 = score*u and A = h*e2 only depend on the gate phase, so
                # they run here, off the post-tanh critical tail.
                e2 = wpool.tile([128, 2, NB], FP32, tag="e2")
                nc.vector.tensor_mul(e2[:], u[:], grp["sbc"][:, :, js])
                A = wpool.tile([128, 2, NB], FP32, tag="A")
                nc.vector.tensor_mul(A[:], ht, e2[:])
                return dict(b=b, j=j, grp=grp, xt=xt, rh=rh, e2=e2, A=A)

            def emit_h(st):
                """h_hat matmul + tanh + final combine + store for block b."""
                b = st["b"]
                if psum_fine:
                    phs = [phpool.tile([128, NB], FP32, tag="ph", name=f"ph{b}_{i}") for i in range(2)]
                else:
                    ph = phpool.tile([128, 2, NB], FP32, tag="ph")
                    phs = [ph[:, 0, :], ph[:, 1, :]]
                for m in range(2):
                    for k in range(4):
                        act = st["xt"][:, k] if k < 2 else st["rh"][:, k - 2, :]
                        c = m * 4 + k
                        nc.tensor.matmul(
                            phs[m][:] if psum_fine else phs[m],
                            wh_sb[:, c * 128:(c + 1) * 128],
                            act,
                            start=(k == 0),
                            stop=(k == 3),
                        )
                hhat = wpool.tile([128, 2, NB], FP32, tag="hhat")
                for m in range(2):
                    nc.scalar.activation(
                        hhat[:, m, :], phs[m][:] if psum_fine else phs[m],
                        AF.Tanh, bias=bh_sb[:, m:m + 1]
                    )
                # out = A - (e2-1)*hh  ==  hh + e2*(h - hh), with A = h*e2
                C = wpool.tile([128, 2, NB], FP32, tag="C")
                nc.vector.scalar_tensor_tensor(
                    C[:], st["e2"][:], 1.0, hhat[:],
                    op0=mybir.AluOpType.subtract, op1=mybir.AluOpType.mult,
                )
                j = st["j"]
                og = st["grp"]["og"]
                nc.vector.tensor_sub(og[:, :, j * NB:(j + 1) * NB],
                                     st["A"][:], C[:])
                if j == group - 1:
                    # store on the ACT HWDGE ring so it doesn't queue behind
                    # the input loads on the SP ring
                    out_eng = nc.scalar if out_queue == "scalar" else nc.sync
                    out_eng.dma_start(outTr[st["grp"]["g"]], og[:])

            # Software-pipelined emission: block b's h-chain is emitted after
            # block b+1's gate matmuls so the PE never waits on the r*h
            # elementwise product. reps>1 repeats the whole pass (same
            # output) — used only for slope-based timing in bench.py.
            def emit_pass():
                prev = None
                for _rep in range(reps):
                    for g in range(NBLK // group):
                        grp = load_group(g)
                        for j in range(group):
                            st = emit_gates(grp, j)
                            if prev is not None:
                                emit_h(prev)
                            prev = st
                emit_h(prev)

            if loop is None:
                emit_pass()
            else:
                # bench-only: repeat the whole pass `loop` times inside one
                # NEFF execution for slope-based timing.
                with tc.For_i(0, loop, 1):
                    emit_pass()

    nc.compile()
    return nc


def _get_nc():
    if "nc" not in _NC_CACHE:
        _NC_CACHE["nc"] = _build_nc()
    return _NC_CACHE["nc"]


def _pack_weights(W_r, W_u, W_h, b_r, b_u, b_h):
    wg = np.empty((128, 24 * 128), np.float32)
    for gi in range(4):
        W = W_r if gi < 2 else W_u
        m = gi % 2
        for k in range(6):
            c = gi * 6 + k
            wg[:, c * 128:(c + 1) * 128] = W[m * 128:(m + 1) * 128,
                                             k * 128:(k + 1) * 128].T
    wh = np.empty((128, 8 * 128), np.float32)
    for m in range(2):
        for k in range(4):
            c = m * 4 + k
            wh[:, c * 128:(c + 1) * 128] = W_h[m * 128:(m + 1) * 128,
                                               k * 128:(k + 1) * 128].T
    bg = np.stack([b_r[:128], b_r[128:], b_u[:128], b_u[128:]], axis=1)
    bh = np.stack([b_h[:128], b_h[128:]], axis=1)
    return (np.ascontiguousarray(wg), np.ascontiguousarray(wh),
            np.ascontiguousarray(bg), np.ascontiguousarray(bh))


def _make_in_maps(inputs, h_prev, attention_score, W_r, b_r, W_u, b_u, W_h, b_h):
    inputs = np.asarray(inputs, np.float32)
    h_prev = np.asarray(h_prev, np.float32)
    attention_score = np.asarray(attention_score, np.float32)
    wg, wh, bg, bh = _pack_weights(
        np.asarray(W_r, np.float32), np.asarray(W_u, np.float32),
        np.asarray(W_h, np.float32), np.asarray(b_r, np.float32),
        np.asarray(b_u, np.float32), np.asarray(b_h, np.float32),
    )
    in_maps = []
    for c in range(NCORES):
        sl = slice(c * BC, (c + 1) * BC)
        in_maps.append({
            "xT": np.ascontiguousarray(inputs[sl].T),
            "hT": np.ascontiguousarray(h_prev[sl].T),
            "sc": np.ascontiguousarray(attention_score[sl].reshape(NBLK, 1, NB)),
            "wg": wg, "wh": wh, "bg": bg, "bh": bh,
        })
    return in_maps


def _run(in_maps, trace=False, **kwargs):
    return run_bass_kernel_spmd(
        _get_nc(), in_maps, core_ids=list(range(NCORES)), trace=trace, **kwargs
    )


def _gather(results):
    out = np.empty((B, H), np.float32)
    for c in range(NCORES):
        out[c * BC:(c + 1) * BC] = results[c]["outT"].T
    return out


def kernel(**inputs):
    res = _run(_make_in_maps(**inputs), trace=False)
    return _gather(res.results)
